# revision 18
# baseline (speedup 1.0000x reference)
"""Trainium2 Bass kernel: MergedQKVParallelLinearWithLoRA.

out = x @ w_qkv.T + concat_s( lora_expand_s( lora_shrink_s(x)[token's lora] ) )

Strategy (8 NeuronCores, TOKEN-parallel):
  - Tokens are grouped by LoRA id on the host; each core owns a 1024-token
    chunk and computes ALL 6144 output columns for it.  This removes the
    fully-replicated LoRA shrink a column-parallel split pays (each core
    shrinks only its own tokens), at the cost of streaming the full base
    weight per core (48MB bf16, hidden under 690us of PE work).
  - The LoRA groups are ORDERED (8! search) so every 1024-token chunk
    touches at most 2 LoRAs; each core gets 2*G LoRA "slots" (A/B weights
    + per-token 0/1 masks) as DATA, so the instruction stream is identical
    on every core (SPMD) regardless of where group boundaries fall.  Two
    slots' A matrices stack side-by-side in the 128-wide PE array, so a
    2-slot shrink/expand costs the same PE time as 1 slot.
  - All matmuls run in bf16 (fp32 PSUM accumulation, ~2e-3 rel err vs the
    2e-2 gate): same PE rate as fp32r, half the DMA bytes, FWL-fast
    LDWEIGHTS.
  - Per 128-col output chunk oc: psum[128, 1024] accumulates the K=4096
    base matmul (w chunk stationary, x moving) then the LoRA expand
    (masked shrink as moving), and is copied out once (split across the
    Scalar and Vector engines).
  - x lands as eight 4-k-slice tiles so the shrink k-loop starts as soon
    as the first slice arrives; the first two ocs' k-loops are interleaved
    with the shrink to keep the PE busy during the x/w DMA fill.

kernel() re-derives the slot count G from token_lora_idx on every call, so
it is correct for arbitrary inputs of the fixed shapes below.
"""

import itertools

import ml_dtypes
import numpy as np

import concourse.mybir as mybir
import concourse.tile as tile
from concourse import bacc, bass_utils

T, D = 8192, 4096
L, R = 8, 16
OUT_SLICES = (4096, 1024, 1024)
O = sum(OUT_SLICES)          # 6144
NCORES = 8
TC = T // NCORES             # 1024 tokens per core
P = 128
KT = D // P                  # 32 k-tiles
OC = O // P                  # 48 output-column chunks of 128
RC = 3 * R                   # 48 stacked lora-rank rows (q,k,v)
SLOTP = P                    # 2x48 slot rows padded to 128
HTC = TC // 2                # 512: psum-bank-sized half of the token dim
XCH = 4                      # k-slices per x tile (separate DMA/dep units)
NXT = KT // XCH              # 8 x tiles

F32 = mybir.dt.float32
BF16 = mybir.dt.bfloat16
NPBF16 = ml_dtypes.bfloat16

LAST_RESULT = None           # BassKernelResults of the most recent run


def _order_loras(counts):
    """Order the lora groups so the max #groups overlapping any 1024-token
    chunk is minimized (8! brute force, ~40k orders)."""
    present = [l for l in range(L) if counts[l] > 0]
    best, best_ms = list(range(L)), 10**9
    bounds = [(c * TC, (c + 1) * TC) for c in range(NCORES)]
    for order in itertools.permutations(present):
        p = 0
        maxseg = 0
        # segments per chunk via interval overlap
        segs = [0] * NCORES
        ok = True
        for l in order:
            q = p + counts[l]
            c0, c1 = p // TC, (q - 1) // TC
            for c in range(c0, c1 + 1):
                segs[c] += 1
                if segs[c] > maxseg:
                    maxseg = segs[c]
            p = q
            if maxseg >= best_ms:
                ok = False
                break
        if ok and maxseg < best_ms:
            best_ms, best = maxseg, list(order)
            if best_ms <= 2:
                break
    return best


def _core_segments(ordered_idx):
    """Per-core list of (lora, a, b) token sub-ranges (a/b rel. to chunk)."""
    out = []
    for c in range(NCORES):
        win = ordered_idx[c * TC : (c + 1) * TC]
        segs = []
        a = 0
        for i in range(1, TC + 1):
            if i == TC or win[i] != win[a]:
                segs.append((int(win[a]), a, i))
                a = i
        out.append(segs)
    return out


def _build(G):
    nc = bacc.Bacc("TRN2", target_bir_lowering=False, debug=False,
                   num_devices=NCORES)
    d_x = nc.dram_tensor("xT", [NXT, P, XCH, TC], BF16, kind="ExternalInput")
    d_w = nc.dram_tensor("wT", [OC, P, KT, P], BF16, kind="ExternalInput")
    d_a = nc.dram_tensor("aT", [G, P, KT, SLOTP], BF16, kind="ExternalInput")
    d_b = nc.dram_tensor("B", [G, SLOTP, O], BF16, kind="ExternalInput")
    d_m = nc.dram_tensor("M", [G, SLOTP, TC], BF16, kind="ExternalInput")
    d_o = nc.dram_tensor("out", [O, TC], F32, kind="ExternalOutput")

    # PSUM budget (8 banks of 512 f32): every psum tile here is 2 banks.
    # spsum holds G named shrink tiles (bufs=1); bpsum cycles n_po "po"
    # slots shared by the early ocs and the main loop.
    n_po = 3 if G == 1 else 2
    n_early = n_po if G <= 2 else 0

    with tile.TileContext(nc) as tc:
        with (
            tc.tile_pool(name="xpool", bufs=1) as xpool,
            tc.tile_pool(name="cpool", bufs=1) as cpool,
            tc.tile_pool(name="wpool", bufs=4) as wpool,
            tc.tile_pool(name="wepool", bufs=1) as wepool,
            tc.tile_pool(name="opool", bufs=3) as opool,
            tc.tile_pool(name="bpsum", bufs=n_po, space="PSUM") as bpsum,
            tc.tile_pool(name="spsum", bufs=1, space="PSUM") as spsum,
        ):
            at = [cpool.tile([P, KT, SLOTP], BF16, name=f"at{g}")
                  for g in range(G)]
            bt = [cpool.tile([SLOTP, O], BF16, name=f"bt{g}")
                  for g in range(G)]
            mt = [cpool.tile([SLOTP, TC], BF16, name=f"mt{g}")
                  for g in range(G)]
            sbs = [cpool.tile([SLOTP, TC], BF16, name=f"sb{g}")
                   for g in range(G)]
            xts = [xpool.tile([P, XCH, TC], BF16, name=f"x{i}")
                   for i in range(NXT)]
            wts_e = [wepool.tile([P, KT, P], BF16, name=f"wte{i}")
                     for i in range(n_early)]

            # DMA issue order = arrival order: everything the interleaved
            # k-loop needs first, then the rest.
            # Arrival-ordered prefix: the PE's first microseconds only need
            # x0 + w0 (+A for the trailing shrink); the rest of x outruns
            # the PE's 4-stream consumption pace after that.
            nc.sync.dma_start(xts[0][:], d_x[0])
            if n_early > 0:
                nc.sync.dma_start(wts_e[0][:], d_w[0])
            for g in range(G):
                nc.sync.dma_start(at[g][:], d_a[g])
            for i in range(1, n_early):
                nc.sync.dma_start(wts_e[i][:], d_w[i])
            for i in range(1, NXT):
                nc.sync.dma_start(xts[i][:], d_x[i])
            for g in range(G):
                nc.sync.dma_start(mt[g][:], d_m[g])
            for g in range(G):
                nc.sync.dma_start(bt[g][:], d_b[g])

            def xk(k):
                return xts[k // XCH][:, k % XCH, :]

            def base_k(po, wt, k):
                nc.tensor.matmul(po[:, 0:HTC], wt[:, k, :], xk(k)[:, 0:HTC],
                                 start=(k == 0), stop=False)
                nc.tensor.matmul(po[:, HTC:TC], wt[:, k, :], xk(k)[:, HTC:TC],
                                 start=(k == 0), stop=False)

            def finish_oc(oc, po):
                for g in range(G):
                    last = g == G - 1
                    bsl = bt[g][:, oc * P : (oc + 1) * P]
                    nc.tensor.matmul(po[:, 0:HTC], bsl, sbs[g][:, 0:HTC],
                                     start=False, stop=last)
                    nc.tensor.matmul(po[:, HTC:TC], bsl, sbs[g][:, HTC:TC],
                                     start=False, stop=last)
                ob_a = opool.tile([P, HTC], F32, tag="oba")
                ob_b = opool.tile([P, HTC], F32, tag="obb")
                nc.scalar.activation(ob_a[:], po[:, 0:HTC],
                                     mybir.ActivationFunctionType.Copy)
                nc.vector.tensor_copy(ob_b[:], po[:, HTC:TC])
                nc.sync.dma_start(d_o[oc * P : (oc + 1) * P, 0:HTC], ob_a[:])
                nc.sync.dma_start(d_o[oc * P : (oc + 1) * P, HTC:TC], ob_b[:])

            if G <= 2:
                # shrink + first ocs, interleaved and STAGGERED per k: oc_i
                # trails by 2(i+1) k-steps and the shrink trails last, so the
                # leading stream only ever waits on x while w1.. and A are
                # still in flight.
                pss = [spsum.tile([SLOTP, TC], F32, name=f"ps{g}")
                       for g in range(G)]
                pos_e = [bpsum.tile([P, TC], F32, tag="po", name=f"poe{i}")
                         for i in range(n_early)]
                lag_s = 2 * n_early  # shrink lag
                for j in range(KT + lag_s + 1):
                    for i in range(n_early):
                        k = j - 2 * i
                        if 0 <= k < KT:
                            base_k(pos_e[i], wts_e[i], k)
                    k = j - lag_s
                    if 0 <= k < KT:
                        for g in range(G):
                            nc.tensor.matmul(pss[g][:, 0:HTC], at[g][:, k, :],
                                             xk(k)[:, 0:HTC],
                                             start=(k == 0),
                                             stop=(k == KT - 1))
                            nc.tensor.matmul(pss[g][:, HTC:TC], at[g][:, k, :],
                                             xk(k)[:, HTC:TC],
                                             start=(k == 0),
                                             stop=(k == KT - 1))
                for g in range(G):
                    nc.vector.tensor_tensor(sbs[g][:], pss[g][:], mt[g][:],
                                            mybir.AluOpType.mult)
                for i in range(n_early):
                    finish_oc(i, pos_e[i])
            else:
                # rare fallback (>4 loras in one chunk): sequential shrink
                for g in range(G):
                    ps = spsum.tile([SLOTP, TC], F32, tag="ps")
                    for k in range(KT):
                        nc.tensor.matmul(ps[:, 0:HTC], at[g][:, k, :],
                                         xk(k)[:, 0:HTC],
                                         start=(k == 0), stop=(k == KT - 1))
                        nc.tensor.matmul(ps[:, HTC:TC], at[g][:, k, :],
                                         xk(k)[:, HTC:TC],
                                         start=(k == 0), stop=(k == KT - 1))
                    nc.vector.tensor_tensor(sbs[g][:], ps[:], mt[g][:],
                                            mybir.AluOpType.mult)

            for oc in range(n_early, OC):
                wt = wpool.tile([P, KT, P], BF16, tag="wt")
                nc.sync.dma_start(wt[:], d_w[oc])
                po = bpsum.tile([P, TC], F32, tag="po")
                for k in range(KT):
                    base_k(po, wt, k)
                finish_oc(oc, po)

    nc.compile()
    return nc


def _prep(x, w_qkv, lora_a, lora_b_q, lora_b_k, lora_b_v, perm, core_segs, G):
    # xT[c][i, p, j, t] = x[perm[c*TC+t], (i*XCH+j)*128+p]
    xs = x[perm].astype(NPBF16)
    x_shards = [
        np.ascontiguousarray(
            xs[c * TC : (c + 1) * TC].T.reshape(NXT, XCH, P, TC)
            .transpose(0, 2, 1, 3)
        )
        for c in range(NCORES)
    ]
    # wT[oc, p, k, c] = w_qkv[oc*128+c, k*128+p]  (same for every core)
    w_re = np.ascontiguousarray(
        w_qkv.astype(NPBF16).T.reshape(KT, P, OC, P).transpose(2, 1, 0, 3)
    )
    # aT_all[l][p, k, rc] = lora_a[s, l, r, k*128+p],  rc = 16*s + r
    a_cat = np.ascontiguousarray(
        lora_a.transpose(1, 0, 2, 3)
    ).reshape(L, RC, D).astype(NPBF16)
    aT_all = np.ascontiguousarray(
        a_cat.transpose(2, 0, 1).reshape(KT, P, L, RC).transpose(2, 1, 0, 3)
    )  # [L, P, KT, RC]
    # Zero-padded B: rows 16s..16s+16 only hit slice-s columns.
    bfull = np.zeros((L, RC, O), NPBF16)
    off = 0
    for s, (bs, osz) in enumerate(
        zip((lora_b_q, lora_b_k, lora_b_v), OUT_SLICES)
    ):
        bfull[:, 16 * s : 16 * (s + 1), off : off + osz] = (
            bs.transpose(0, 2, 1).astype(NPBF16)
        )
        off += osz

    a_sh, b_sh, m_sh = [], [], []
    for c in range(NCORES):
        a_c = np.zeros((G, P, KT, SLOTP), NPBF16)
        b_c = np.zeros((G, SLOTP, O), NPBF16)
        m_c = np.zeros((G, SLOTP, TC), NPBF16)
        for j, (l, a, b) in enumerate(core_segs[c]):
            g, lane = j // 2, j % 2
            a_c[g, :, :, lane * RC : (lane + 1) * RC] = aT_all[l]
            b_c[g, lane * RC : (lane + 1) * RC, :] = bfull[l]
            m_c[g, lane * RC : (lane + 1) * RC, a:b] = 1.0
        a_sh.append(a_c)
        b_sh.append(b_c)
        m_sh.append(m_c)
    return x_shards, w_re, a_sh, b_sh, m_sh


def kernel(x, w_qkv, lora_a, lora_b_q, lora_b_k, lora_b_v, token_lora_idx):
    global LAST_RESULT
    idx = np.asarray(token_lora_idx)
    counts = np.bincount(idx, minlength=L)
    order = _order_loras(counts)
    perm = np.concatenate(
        [np.flatnonzero(idx == l) for l in order if counts[l] > 0]
    )
    core_segs = _core_segments(idx[perm])
    G = (max(len(s) for s in core_segs) + 1) // 2

    nc = _build(G)
    x_shards, w_re, a_sh, b_sh, m_sh = _prep(
        np.asarray(x, dtype=np.float32), np.asarray(w_qkv, dtype=np.float32),
        np.asarray(lora_a), np.asarray(lora_b_q), np.asarray(lora_b_k),
        np.asarray(lora_b_v), perm, core_segs, G,
    )
    in_maps = [
        {"xT": x_shards[c], "wT": w_re, "aT": a_sh[c], "B": b_sh[c],
         "M": m_sh[c]}
        for c in range(NCORES)
    ]
    res = bass_utils.run_bass_kernel_spmd(
        nc, in_maps, core_ids=list(range(NCORES))
    )
    LAST_RESULT = res
    out_sorted = np.concatenate(
        [res.results[c]["out"] for c in range(NCORES)], axis=1
    )  # [O, T] in grouped-token order
    out = np.empty((T, O), np.float32)
    out[perm] = out_sorted.T
    return out
